# revision 1
# baseline (speedup 1.0000x reference)
"""Multi-head attention (B=4, S=2048, E=768, H=12) on 8 trn2 NeuronCores.

Sharding: 2-D (batch x head-half). Core c handles batch c//2, heads
(c%2)*6 .. (c%2)*6+5  (Wq/Wk/Wv column-split, Wo row-split). Each core
returns a partial O^T [768, S]; host sums the two head-halves per batch,
transposes, and adds the effective output bias (bo + bv@Wo — softmax rows
sum to 1, so V's bias contributes a constant row folded on the host).

Device kernel (per core), bf16 matmuls + fp32 PSUM:
  - masked keys are compacted away on host; padded keys get -30000 added
    via the exp's per-partition bias -> exp == 0.
  - scores/ctx computed transposed (S^T tiles [128 k, q]) so P^T feeds the
    context matmul directly; V carries an appended ones column so row 64
    of the context accumulator is the softmax denominator.
  - normalization: denominator rows collected (DMA) into 32-aligned rows
    of collector tiles, reciprocal_approx_fast (batched), hi/lo bf16
    split, ones-outer-product broadcast matmul (K=2, exact), multiply.
  - consecutive matmuls always target alternating PSUM banks (same-bank
    back-to-back runs at half rate).
"""

import os
import numpy as np
import ml_dtypes

E = 768
H = 12
D = 64
HALF = 384  # E // 2 output cols per head-half
N_CORES = 8

_CACHE = {}
_LAST = None  # last BassKernelResults (for test harness introspection)

bf16_np = ml_dtypes.bfloat16


def _build(S_q, S_pad):
    from contextlib import ExitStack
    import concourse.bass as bass
    import concourse.tile as tile
    from concourse import bacc, mybir

    bf16 = mybir.dt.bfloat16
    f32 = mybir.dt.float32
    FT = mybir.ActivationFunctionType

    NKC = S_pad // 128
    NMC = HALF // 128        # 3 proj-dim chunks (head pairs)
    NEC = E // 128           # 6 embed chunks
    QB = 512 if S_q % 512 == 0 else S_q
    NQB = S_q // QB
    NU = 6 * NQB             # normalization units
    NCOLL = (NU + 3) // 4    # collector tiles (4 rows each at 0/32/64/96)

    def ntiles(total, step=512):
        return [(s, min(step, total - s)) for s in range(0, total, step)]

    nc = bacc.Bacc("TRN2", target_bir_lowering=False, debug=False,
                   num_devices=N_CORES)

    qT = nc.dram_tensor("qT", [E, S_q], bf16, kind="ExternalInput").ap()
    kT = nc.dram_tensor("kT", [E, S_pad], bf16, kind="ExternalInput").ap()
    vT = nc.dram_tensor("vT", [E, S_pad], bf16, kind="ExternalInput").ap()
    wq = nc.dram_tensor("wq", [E, HALF], bf16, kind="ExternalInput").ap()
    wk = nc.dram_tensor("wk", [E, HALF], bf16, kind="ExternalInput").ap()
    wv = nc.dram_tensor("wv", [E, HALF], bf16, kind="ExternalInput").ap()
    wo = nc.dram_tensor("wo", [HALF, E], bf16, kind="ExternalInput").ap()
    bq2 = nc.dram_tensor("bq2", [128, NMC], f32, kind="ExternalInput").ap()
    bk2 = nc.dram_tensor("bk2", [128, NMC], f32, kind="ExternalInput").ap()
    kbias = nc.dram_tensor("kbias", [128, NKC], f32, kind="ExternalInput").ap()
    oT = nc.dram_tensor("oT", [E, S_q], f32, kind="ExternalOutput").ap()

    with tile.TileContext(nc) as tc, ExitStack() as ctx:
        cons = ctx.enter_context(tc.tile_pool(name="cons", bufs=1))
        wp = ctx.enter_context(tc.tile_pool(name="wp", bufs=1))
        acts = ctx.enter_context(tc.tile_pool(name="acts", bufs=1))
        pp = ctx.enter_context(tc.tile_pool(name="pp", bufs=6))
        ost = ctx.enter_context(tc.tile_pool(name="ost", bufs=4))
        nrm = ctx.enter_context(tc.tile_pool(name="nrm", bufs=1))

        # ---- constant/small loads ----
        bq2_t = cons.tile([128, NMC], f32, tag="bq2")
        bk2_t = cons.tile([128, NMC], f32, tag="bk2")
        kb_t = cons.tile([128, NKC], f32, tag="kb")
        ones2 = cons.tile([2, 64], bf16, tag="ones2")
        nc.sync.dma_start(bq2_t[:], bq2[:])
        nc.sync.dma_start(bk2_t[:], bk2[:])
        nc.sync.dma_start(kb_t[:], kbias[:])
        nc.vector.memset(ones2[:], 1.0)

        # ---- weight + input loads (inputs in a scoped pool, freed after proj)
        qkv = tc.tile_pool(name="qkv", bufs=1)
        inp = qkv.__enter__()
        wq_t = [wp.tile([128, HALF], bf16, tag=f"wq{e}", name=f"wq{e}") for e in range(NEC)]
        wk_t = [wp.tile([128, HALF], bf16, tag=f"wk{e}", name=f"wk{e}") for e in range(NEC)]
        wv_t = [wp.tile([128, HALF], bf16, tag=f"wv{e}", name=f"wv{e}") for e in range(NEC)]
        wo_t = [wp.tile([128, E], bf16, tag=f"wo{m}", name=f"wo{m}") for m in range(NMC)]
        kT_t = [inp.tile([128, S_pad], bf16, tag=f"kT{e}", name=f"kTt{e}") for e in range(NEC)]
        vT_t = [inp.tile([128, S_pad], bf16, tag=f"vT{e}", name=f"vTt{e}") for e in range(NEC)]
        qT_t = [inp.tile([128, S_q], bf16, tag=f"qT{e}", name=f"qTt{e}") for e in range(NEC)]
        for e in range(NEC):
            nc.sync.dma_start(wk_t[e][:], wk[128 * e:128 * (e + 1), :])
            nc.sync.dma_start(kT_t[e][:], kT[128 * e:128 * (e + 1), :])
        for e in range(NEC):
            nc.sync.dma_start(wv_t[e][:], wv[128 * e:128 * (e + 1), :])
            nc.sync.dma_start(vT_t[e][:], vT[128 * e:128 * (e + 1), :])
        for e in range(NEC):
            nc.sync.dma_start(wq_t[e][:], wq[128 * e:128 * (e + 1), :])
            nc.sync.dma_start(qT_t[e][:], qT[128 * e:128 * (e + 1), :])
        for m in range(NMC):
            nc.sync.dma_start(wo_t[m][:], wo[128 * m:128 * (m + 1), :])

        # ---- projections (pairs of output tiles -> alternating PSUM banks)
        kts = [acts.tile([128, S_pad], bf16, tag=f"kts{m}", name=f"kts{m}") for m in range(NMC)]
        qts = [acts.tile([128, S_q], bf16, tag=f"qts{m}", name=f"qts{m}") for m in range(NMC)]
        vhx = [acts.tile([128, 6, 128], bf16, tag=f"vhx{j}", name=f"vhx{j}") for j in range(NKC)]

        psp = tc.tile_pool(name="psp", bufs=1, space="PSUM")
        ps = psp.__enter__()

        def proj_kq(wt, xt, out, bias_t, total):
            # out^T[m-chunk, n] accumulated over NEC embed chunks; n-tiles
            # processed in pairs so consecutive matmuls alternate banks.
            for m in range(NMC):
                tiles = ntiles(total)
                for i in range(0, len(tiles), 2):
                    pair = tiles[i:i + 2]
                    pjs = [ps.tile([128, 512], f32, tag=f"pj{j}", bufs=2,
                                   name=f"pj_{m}_{i}_{j}")
                           for j in range(len(pair))]
                    for e in range(NEC):
                        for j, (n0, nw) in enumerate(pair):
                            nc.tensor.matmul(
                                pjs[j][:, :nw],
                                wt[e][:, 128 * m:128 * (m + 1)],
                                xt[e][:, n0:n0 + nw],
                                start=(e == 0), stop=(e == NEC - 1))
                    for j, (n0, nw) in enumerate(pair):
                        nc.scalar.activation(out[m][:, n0:n0 + nw],
                                             pjs[j][:, :nw],
                                             FT.Identity,
                                             bias=bias_t[:, m:m + 1])

        proj_kq(wk_t, kT_t, kts, bk2_t, S_pad)

        # V projection: natural layout, s-chunk pairs
        for i in range(0, NKC, 2):
            js = [j for j in (i, i + 1) if j < NKC]
            pvs = [ps.tile([128, HALF], f32, tag=f"pv{j - i}", bufs=2,
                           name=f"pv{j}") for j in js]
            for e in range(NEC):
                for x, j in enumerate(js):
                    nc.tensor.matmul(pvs[x][:],
                                     vT_t[e][:, 128 * j:128 * (j + 1)],
                                     wv_t[e][:],
                                     start=(e == 0), stop=(e == NEC - 1))
            for x, j in enumerate(js):
                nc.vector.memset(vhx[j][:, :, 64:128], 1.0)
                nc.scalar.copy(vhx[j][:, :, 0:64],
                               pvs[x][:].rearrange("p (h d) -> p h d", h=6))

        proj_kq(wq_t, qT_t, qts, bq2_t, S_q)
        psp.__exit__(None, None, None)
        qkv.__exit__(None, None, None)

        # ---- attention ----
        czT = [acts.tile([128, S_q], bf16, tag=f"czT{m}", name=f"czT{m}") for m in range(NMC)]
        den_t = [nrm.tile([97, QB], f32, tag=f"den{t}", name=f"den{t}")
                 for t in range(NCOLL)]
        for t in range(NCOLL):
            nc.vector.memset(den_t[t][:], 1.0)

        psa = tc.tile_pool(name="psa", bufs=1, space="PSUM")
        ps = psa.__enter__()

        deferred = []
        hilo = {}

        def make_group(t):
            def group():
                recq = nrm.tile([97, QB], f32, tag="recq", bufs=2,
                                name=f"recq{t}")
                nc.vector.reciprocal_approx_fast(recq[:], den_t[t][:])
                hi_t = nrm.tile([97, QB], bf16, tag="hi", bufs=2,
                                name=f"hi{t}")
                lo_t = nrm.tile([97, QB], bf16, tag="lo", bufs=2,
                                name=f"lo{t}")
                nc.vector.tensor_copy(hi_t[:], recq[:])
                nc.vector.tensor_sub(lo_t[:], recq[:], hi_t[:])
                hilo[t] = (hi_t, lo_t)
            return group

        def make_unit(u, cs):
            qb, h = divmod(u, 6)
            m, half = divmod(h, 2)
            t, r = divmod(u, 4)

            def unit():
                hi_t, lo_t = hilo[t]
                hl = nrm.tile([2, QB], bf16, tag="hl", bufs=4, name=f"hl{u}")
                nc.sync.dma_start(hl[0:1, :], hi_t[32 * r:32 * r + 1, :])
                nc.sync.dma_start(hl[1:2, :], lo_t[32 * r:32 * r + 1, :])
                bcp = ps.tile([64, QB], f32, tag="bc", bufs=2, name=f"bcp{u}")
                for (t0, tw) in ntiles(QB):
                    nc.tensor.matmul(bcp[:, t0:t0 + tw], ones2[:],
                                     hl[:, t0:t0 + tw], start=True, stop=True)
                nc.vector.tensor_mul(
                    czT[m][64 * half:64 * (half + 1), qb * QB:(qb + 1) * QB],
                    cs[0:64, :], bcp[:])
            return unit

        group_units = {}

        def evac(u, C):
            cs = nrm.tile([65, QB], f32, tag="cs", bufs=5, name=f"cs{u}")
            nc.vector.tensor_copy(cs[:], C[0:65, :])
            t, r = divmod(u, 4)
            nc.sync.dma_start(den_t[t][32 * r:32 * r + 1, :], cs[64:65, :])
            group_units.setdefault(t, []).append(make_unit(u, cs))
            if r == 3 or u == NU - 1:
                deferred.append(make_group(t))
                deferred.extend(group_units.pop(t))

        pend_cs = []  # [(u, C), ...] awaiting evacuation
        for qb in range(NQB):
            q0 = qb * QB
            for p in range(NMC):  # head pair: hA=2p (rows 0-63), hB=2p+1
                hA, hB = 2 * p, 2 * p + 1
                CA = ps.tile([128, QB], f32, tag="CA", name=f"CA{qb}_{p}")
                CB = ps.tile([128, QB], f32, tag="CB", name=f"CB{qb}_{p}")
                for pc in pend_cs:
                    evac(*pc)
                pend_cs = []

                def sc_pair(kc, SA_t, SB_t):
                    nc.tensor.matmul(
                        SA_t[:], kts[p][0:64, 128 * kc:128 * (kc + 1)],
                        qts[p][0:64, q0:q0 + QB],
                        start=True, stop=True, tile_position=(0, 0))
                    nc.tensor.matmul(
                        SB_t[:], kts[p][64:128, 128 * kc:128 * (kc + 1)],
                        qts[p][64:128, q0:q0 + QB],
                        start=True, stop=True, tile_position=(64, 0))

                SA = ps.tile([128, QB], f32, tag="SA", bufs=2,
                             name=f"SA{qb}_{p}_0")
                SB = ps.tile([128, QB], f32, tag="SB", bufs=2,
                             name=f"SB{qb}_{p}_0")
                sc_pair(0, SA, SB)
                for kc in range(NKC):
                    if kc >= 1 and deferred:
                        deferred.pop(0)()
                    SA2 = SB2 = None
                    if kc + 1 < NKC:
                        SA2 = ps.tile([128, QB], f32, tag="SA", bufs=2,
                                      name=f"SA{qb}_{p}_{kc + 1}")
                        SB2 = ps.tile([128, QB], f32, tag="SB", bufs=2,
                                      name=f"SB{qb}_{p}_{kc + 1}")
                        sc_pair(kc + 1, SA2, SB2)
                    PA = pp.tile([128, QB], bf16, tag="P", name=f"PA{qb}_{p}_{kc}")
                    PB = pp.tile([128, QB], bf16, tag="P", name=f"PB{qb}_{p}_{kc}")
                    nc.scalar.activation(PA[:], SA[:], FT.Exp,
                                         bias=kb_t[:, kc:kc + 1], scale=1.0)
                    nc.scalar.activation(PB[:], SB[:], FT.Exp,
                                         bias=kb_t[:, kc:kc + 1], scale=1.0)
                    nc.tensor.matmul(CA[:], vhx[kc][:, hA, :], PA[:],
                                     start=(kc == 0), stop=(kc == NKC - 1))
                    nc.tensor.matmul(CB[:], vhx[kc][:, hB, :], PB[:],
                                     start=(kc == 0), stop=(kc == NKC - 1))
                    SA, SB = SA2, SB2
                pend_cs = [(qb * 6 + hA, CA), (qb * 6 + hB, CB)]

        # flush: evacuate last heads, then drain deferred queue
        for pc in pend_cs:
            evac(*pc)
        for fn in deferred:
            fn()
        psa.__exit__(None, None, None)

        # ---- output projection: O^T[e-chunk, q] = sum_m wo_t[m].T @ czT[m]
        pso = tc.tile_pool(name="pso", bufs=1, space="PSUM")
        ps = pso.__enter__()
        tiles_o = [(ec, t0, tw) for ec in range(NEC)
                   for (t0, tw) in ntiles(S_q)]
        for i in range(0, len(tiles_o), 2):
            pair = tiles_o[i:i + 2]
            pos = [ps.tile([128, 512], f32, tag=f"po{j}", bufs=2,
                           name=f"po{i}_{j}") for j in range(len(pair))]
            for mm in range(NMC):
                for j, (ec, t0, tw) in enumerate(pair):
                    nc.tensor.matmul(pos[j][:, :tw],
                                     wo_t[mm][:, 128 * ec:128 * (ec + 1)],
                                     czT[mm][:, t0:t0 + tw],
                                     start=(mm == 0), stop=(mm == NMC - 1))
            for j, (ec, t0, tw) in enumerate(pair):
                ot = ost.tile([128, 512], f32, tag="ot", name=f"ot{i}_{j}")
                nc.scalar.copy(ot[:, :tw], pos[j][:, :tw])
                nc.sync.dma_start(oT[128 * ec:128 * (ec + 1), t0:t0 + tw],
                                  ot[:, :tw])
        pso.__exit__(None, None, None)

    nc.compile()
    return nc


def _numpy_fallback(q, k, v, mask, Wq, bq, Wk, bk, Wv, bv, Wo, bo):
    B, Sq, _ = q.shape
    qh = (q @ Wq + bq).reshape(B, Sq, H, D).transpose(0, 2, 1, 3)
    kh = (k @ Wk + bk).reshape(B, -1, H, D).transpose(0, 2, 1, 3)
    vh = (v @ Wv + bv).reshape(B, -1, H, D).transpose(0, 2, 1, 3)
    s = np.einsum("bhqd,bhkd->bhqk", qh, kh) / np.sqrt(np.float32(D))
    s = s + np.where(mask == 0, np.float32(-1e9), np.float32(0))[:, None, None, :]
    s = s - s.max(-1, keepdims=True)
    w = np.exp(s)
    w = w / w.sum(-1, keepdims=True)
    ctx = np.einsum("bhqk,bhkd->bqhd", w, vh).reshape(B, Sq, E)
    return (ctx @ Wo + bo).astype(np.float32)


def kernel(q, k, v, mask, Wq, bq, Wk, bk, Wv, bv, Wo, bo):
    global _LAST
    q = np.asarray(q, np.float32)
    k = np.asarray(k, np.float32)
    v = np.asarray(v, np.float32)
    mask = np.asarray(mask)
    Wq = np.asarray(Wq, np.float32)
    bq = np.asarray(bq, np.float32)
    Wk = np.asarray(Wk, np.float32)
    bk = np.asarray(bk, np.float32)
    Wv = np.asarray(Wv, np.float32)
    bv = np.asarray(bv, np.float32)
    Wo = np.asarray(Wo, np.float32)
    bo = np.asarray(bo, np.float32)

    B, S_q, _ = q.shape
    idxs = [np.flatnonzero(mask[b]) for b in range(B)]
    ns = [len(ix) for ix in idxs]
    if min(ns) == 0 or B * 2 != N_CORES or S_q % 512 != 0:
        return _numpy_fallback(q, k, v, mask, Wq, bq, Wk, bk, Wv, bv, Wo, bo)

    S_pad = max(128, ((max(ns) + 127) // 128) * 128)
    NKC = S_pad // 128
    NMC = HALF // 128

    key = (S_q, S_pad)
    if key not in _CACHE:
        _CACHE[key] = _build(S_q, S_pad)
    nc = _CACHE[key]

    scale = np.float32(1.0 / np.sqrt(D))
    in_maps = []
    for c in range(N_CORES):
        b, j = divmod(c, 2)
        cols = slice(j * HALF, (j + 1) * HALF)
        kc_ = np.zeros((S_pad, E), np.float32)
        kc_[:ns[b]] = k[b][idxs[b]]
        vc_ = np.zeros((S_pad, E), np.float32)
        vc_[:ns[b]] = v[b][idxs[b]]
        kb_vec = np.zeros(S_pad, np.float32)
        kb_vec[ns[b]:] = -30000.0
        in_maps.append({
            "qT": np.ascontiguousarray(q[b].T).astype(bf16_np),
            "kT": np.ascontiguousarray(kc_.T).astype(bf16_np),
            "vT": np.ascontiguousarray(vc_.T).astype(bf16_np),
            "wq": (Wq[:, cols] * scale).astype(bf16_np),
            "wk": np.ascontiguousarray(Wk[:, cols]).astype(bf16_np),
            "wv": np.ascontiguousarray(Wv[:, cols]).astype(bf16_np),
            "wo": np.ascontiguousarray(Wo[cols, :]).astype(bf16_np),
            "bq2": np.ascontiguousarray((bq[cols] * scale).reshape(NMC, 128).T),
            "bk2": np.ascontiguousarray(bk[cols].reshape(NMC, 128).T),
            "kbias": np.ascontiguousarray(kb_vec.reshape(NKC, 128).T),
        })

    from concourse.bass_utils import run_bass_kernel_spmd
    res = run_bass_kernel_spmd(nc, in_maps, list(range(N_CORES)))
    _LAST = res

    bo_eff = bo + bv @ Wo
    out = np.empty((B, S_q, E), np.float32)
    for b in range(B):
        out[b] = (res.results[2 * b]["oT"] + res.results[2 * b + 1]["oT"]).T
        out[b] += bo_eff
    return out



# revision 39
# speedup vs baseline: 1.6307x; 1.6307x over previous
"""Multi-head attention (B=4, S=2048, E=768, H=12) on 8 trn2 NeuronCores.

Sharding: 2-D (batch x head-half). Core c handles batch c//2, heads
(c%2)*6 .. (c%2)*6+5  (Wq/Wk/Wv column-split, Wo row-split). Each core
returns a partial O^T [768, S]; host sums the two head-halves per batch,
transposes, and adds the effective output bias (bo + bv@Wo — softmax rows
sum to 1, so V's bias contributes a constant row folded on the host).

Device kernel (per core), bf16 matmuls + fp32 PSUM:
  - masked keys are compacted away on host; padded keys get -30000 added
    via the exp's per-partition bias -> exp == 0.
  - scores/ctx computed transposed (S^T tiles [128 k, q]) so P^T feeds the
    context matmul directly; V carries an appended ones column so row 64
    of the context accumulator is the softmax denominator.
  - the two heads of a pair share one 2-bank PSUM tile [128, 2*QB], so a
    single wide Exp covers both (halves the Act instruction count).
  - normalization: reciprocal_approx_fast straight from the PSUM
    denominator row, then gpsimd partition_broadcast (exact, on the idle
    Pool engine) spreads it across 64 partitions; czT = ctx * bcast on DVE.
  - PE is kept back-logged through the Act-paced attention phase by a
    filler queue (normalization broadcasts + previous q-block's output
    projection), so the tensor engine never idles and holds its 2.4 GHz
    p-state; consecutive matmuls always target different PSUM banks.
"""

import os
import numpy as np
import ml_dtypes

E = 768
H = 12
D = 64
HALF = 384  # E // 2 output cols per head-half
N_CORES = 8

_CACHE = {}
_LAST = None  # last BassKernelResults (for test harness introspection)

bf16_np = ml_dtypes.bfloat16


def _build(S_q, S_pad):
    from collections import deque
    from contextlib import ExitStack
    import concourse.bass as bass
    import concourse.tile as tile
    from concourse import bacc, mybir

    bf16 = mybir.dt.bfloat16
    f32 = mybir.dt.float32
    FT = mybir.ActivationFunctionType

    NKC = S_pad // 128
    NMC = HALF // 128        # 3 proj-dim chunks (head pairs)
    NEC = E // 128           # 6 embed chunks
    QB = 512 if S_q % 512 == 0 else S_q
    NQB = S_q // QB

    def ntiles(total, step=512):
        return [(s, min(step, total - s)) for s in range(0, total, step)]

    nc = bacc.Bacc("TRN2", target_bir_lowering=False, debug=False,
                   num_devices=N_CORES)

    qT = nc.dram_tensor("qT", [E, S_q], bf16, kind="ExternalInput").ap()
    kT = nc.dram_tensor("kT", [E, S_pad], bf16, kind="ExternalInput").ap()
    vT = nc.dram_tensor("vT", [E, S_pad], bf16, kind="ExternalInput").ap()
    wq = nc.dram_tensor("wq", [E, HALF], bf16, kind="ExternalInput").ap()
    wk = nc.dram_tensor("wk", [E, HALF], bf16, kind="ExternalInput").ap()
    wv = nc.dram_tensor("wv", [E, HALF], bf16, kind="ExternalInput").ap()
    wo = nc.dram_tensor("wo", [HALF, E], bf16, kind="ExternalInput").ap()
    bq2 = nc.dram_tensor("bq2", [128, NMC], f32, kind="ExternalInput").ap()
    bk2 = nc.dram_tensor("bk2", [128, NMC], f32, kind="ExternalInput").ap()
    kbias = nc.dram_tensor("kbias", [128, NKC], f32, kind="ExternalInput").ap()
    oT = nc.dram_tensor("oT", [E, S_q], f32, kind="ExternalOutput").ap()

    with tile.TileContext(nc) as tc, ExitStack() as ctx:
        cons = ctx.enter_context(tc.tile_pool(name="cons", bufs=1))
        wp = ctx.enter_context(tc.tile_pool(name="wp", bufs=1))
        acts = ctx.enter_context(tc.tile_pool(name="acts", bufs=1))
        pp = ctx.enter_context(tc.tile_pool(name="pp", bufs=3))
        ost = ctx.enter_context(tc.tile_pool(name="ost", bufs=10))
        nrm = ctx.enter_context(tc.tile_pool(name="nrm", bufs=1))

        # ---- constant/small loads ----
        bq2_t = cons.tile([128, NMC], f32, tag="bq2")
        bk2_t = cons.tile([128, NMC], f32, tag="bk2")
        kb_t = cons.tile([128, NKC], f32, tag="kb")
        nc.sync.dma_start(bq2_t[:], bq2[:])
        nc.sync.dma_start(bk2_t[:], bk2[:])
        nc.sync.dma_start(kb_t[:], kbias[:])

        # ---- weight + input loads (inputs in a scoped pool, freed after proj)
        qkv = tc.tile_pool(name="qkv", bufs=1)
        inp = qkv.__enter__()
        wq_t = [wp.tile([128, HALF], bf16, tag=f"wq{e}", name=f"wq{e}") for e in range(NEC)]
        wk_t = [wp.tile([128, HALF], bf16, tag=f"wk{e}", name=f"wk{e}") for e in range(NEC)]
        wv_t = [wp.tile([128, HALF], bf16, tag=f"wv{e}", name=f"wv{e}") for e in range(NEC)]
        wo_t = [wp.tile([128, E], bf16, tag=f"wo{m}", name=f"wo{m}") for m in range(NMC)]
        kT_t = [inp.tile([128, S_pad], bf16, tag=f"kT{e}", name=f"kTt{e}") for e in range(NEC)]
        vT_t = [inp.tile([128, S_pad], bf16, tag=f"vT{e}", name=f"vTt{e}") for e in range(NEC)]
        qT_t = [inp.tile([128, S_q], bf16, tag=f"qT{e}", name=f"qTt{e}") for e in range(NEC)]
        for e in range(NEC):
            nc.sync.dma_start(wk_t[e][:], wk[128 * e:128 * (e + 1), :])
            nc.sync.dma_start(kT_t[e][:], kT[128 * e:128 * (e + 1), :])
        for e in range(NEC):
            nc.sync.dma_start(wv_t[e][:], wv[128 * e:128 * (e + 1), :])
            nc.sync.dma_start(vT_t[e][:], vT[128 * e:128 * (e + 1), :])
        for e in range(NEC):
            nc.sync.dma_start(wq_t[e][:], wq[128 * e:128 * (e + 1), :])
            nc.sync.dma_start(qT_t[e][:], qT[128 * e:128 * (e + 1), :])
        for m in range(NMC):
            nc.sync.dma_start(wo_t[m][:], wo[128 * m:128 * (m + 1), :])

        # ---- projections (pairs of output tiles -> alternating PSUM banks)
        kts = [acts.tile([128, S_pad], bf16, tag=f"kts{m}", name=f"kts{m}") for m in range(NMC)]
        qts = [acts.tile([128, S_q], bf16, tag=f"qts{m}", name=f"qts{m}") for m in range(NMC)]
        vhx = [acts.tile([128, 6, 128], bf16, tag=f"vhx{j}", name=f"vhx{j}") for j in range(NKC)]

        psp = tc.tile_pool(name="psp", bufs=1, space="PSUM")
        ps = psp.__enter__()

        def proj_kq(wt, xt, out, bias_t, total, ms=None):
            # out^T[m-chunk, n] accumulated over NEC embed chunks; n-tiles
            # processed in pairs so consecutive matmuls alternate banks.
            for m in (range(NMC) if ms is None else ms):
                tiles = ntiles(total)
                for i in range(0, len(tiles), 2):
                    pair = tiles[i:i + 2]
                    pjs = [ps.tile([128, 512], f32, tag=f"pj{j}", bufs=2,
                                   name=f"pj_{m}_{i}_{j}")
                           for j in range(len(pair))]
                    for e in range(NEC):
                        for j, (n0, nw) in enumerate(pair):
                            nc.tensor.matmul(
                                pjs[j][:, :nw],
                                wt[e][:, 128 * m:128 * (m + 1)],
                                xt[e][:, n0:n0 + nw],
                                start=(e == 0), stop=(e == NEC - 1))
                    for j, (n0, nw) in enumerate(pair):
                        nc.scalar.activation(out[m][:, n0:n0 + nw],
                                             pjs[j][:, :nw],
                                             FT.Identity,
                                             bias=bias_t[:, m:m + 1])

        proj_kq(wk_t, kT_t, kts, bk2_t, S_pad)

        # V projection: natural layout, s-chunk pairs
        for i in range(0, NKC, 2):
            js = [j for j in (i, i + 1) if j < NKC]
            pvs = [ps.tile([128, HALF], f32, tag=f"pv{j - i}", bufs=2,
                           name=f"pv{j}") for j in js]
            for e in range(NEC):
                for x, j in enumerate(js):
                    nc.tensor.matmul(pvs[x][:],
                                     vT_t[e][:, 128 * j:128 * (j + 1)],
                                     wv_t[e][:],
                                     start=(e == 0), stop=(e == NEC - 1))
            for x, j in enumerate(js):
                nc.vector.memset(vhx[j][:, :, 64:128], 1.0)
                nc.scalar.copy(vhx[j][:, :, 0:64],
                               pvs[x][:].rearrange("p (h d) -> p h d", h=6))

        proj_kq(wq_t, qT_t, qts, bq2_t, S_q, ms=(0, 1))
        psp.__exit__(None, None, None)

        # ---- attention ----
        czT = [acts.tile([128, S_q], bf16, tag=f"czT{m}", name=f"czT{m}") for m in range(NMC)]

        psa = tc.tile_pool(name="psa", bufs=1, space="PSUM")
        ps = psa.__enter__()

        fillq = deque()

        # allocate the C/fill PSUM tags before S2's first tile so S2 lands
        # on the banks vproj freed long ago rather than the ones the last
        # qproj pair is still evacuating at the phase transition
        ps.tile([128, QB], f32, tag="C", bufs=3, name="Cwarm")
        ps.tile([128, QB], f32, tag="fill", name="fillwarm")

        # qproj m=2 runs as pre-seeded fillers inside the attention phase
        # (needed only from head pair p=2, ~70 iterations in), so the Act
        # exp pipeline starts ~5us earlier and qb0 has PE backlog.
        def make_qprojm(m, n0, nw):
            def mk(e):
                def f():
                    if e == 0:
                        make_qprojm.pj = ps.tile([128, QB], f32, tag="fill",
                                                 name=f"qp{m}_{n0}")
                    pj = make_qprojm.pj
                    nc.tensor.matmul(pj[:, :nw],
                                     wq_t[e][:, 128 * m:128 * (m + 1)],
                                     qT_t[e][:, n0:n0 + nw],
                                     start=(e == 0), stop=(e == NEC - 1))
                    if e == NEC - 1:
                        nc.scalar.activation(qts[m][:, n0:n0 + nw],
                                             pj[:, :nw], FT.Identity,
                                             bias=bq2_t[:, m:m + 1])
                return f
            return [mk(e) for e in range(NEC)]

        for (n0, nw) in ntiles(S_q):
            fillq.extend(make_qprojm(2, n0, nw))

        def evac(u, C):
            # normalization runs entirely off the tensor engine. Partition
            # moves only work via DMA or partition_broadcast-from-partition-0
            # (cross-partition DVE reads silently corrupt), so: copy ctx+den
            # rows to SBUF, DMA the den row to partition 0, reciprocal there,
            # Pool-engine broadcast across 64 partitions, DVE multiply.
            qb, h = divmod(u, 6)
            m, half = divmod(h, 2)
            cs = nrm.tile([65, QB], f32, tag="cs", bufs=4, name=f"cs{u}")
            nc.vector.tensor_copy(cs[:], C[0:65, :])
            dnr = nrm.tile([1, QB], f32, tag="dnr", bufs=4, name=f"dnr{u}")
            nc.sync.dma_start(dnr[:], cs[64:65, :])
            rq = nrm.tile([1, QB], f32, tag="rq", bufs=4, name=f"rq{u}")
            nc.vector.reciprocal_approx_fast(rq[:], dnr[:])
            bcb = nrm.tile([64, QB], f32, tag="bcb", bufs=4, name=f"bcb{u}")
            nc.gpsimd.partition_broadcast(bcb[:], rq[:])
            nc.vector.tensor_mul(
                czT[m][64 * half:64 * (half + 1), qb * QB:(qb + 1) * QB],
                cs[0:64, :], bcb[:])

        def make_outproj(qb, ec):
            # 3 accumulating matmuls into the fill bank + evacuation
            t0 = qb * QB

            def mk(mm):
                def f():
                    if mm == 0:
                        make_outproj.po = ps.tile([128, QB], f32, tag="fill",
                                                  name=f"po{qb}_{ec}")
                    po = make_outproj.po
                    nc.tensor.matmul(po[:],
                                     wo_t[mm][:, 128 * ec:128 * (ec + 1)],
                                     czT[mm][:, t0:t0 + QB],
                                     start=(mm == 0), stop=(mm == NMC - 1))
                    if mm == NMC - 1:
                        ot = ost.tile([128, QB], f32, tag="ot",
                                      name=f"ot{qb}_{ec}")
                        nc.vector.tensor_copy(ot[:], po[:])
                        nc.sync.dma_start(
                            oT[128 * ec:128 * (ec + 1), t0:t0 + QB], ot[:])
                return f
            return [mk(mm) for mm in range(NMC)]

        for qb in range(NQB):
            q0 = qb * QB
            for p in range(NMC):  # head pair: hA=2p (rows 0-63), hB=2p+1
                if qb >= 1 and p == 1:
                    # enqueue one iteration late so the previous q-block's
                    # last normalization (DMA+recip+broadcast chain) lands
                    # before the first filler needs czT[2]
                    for ec in range(NEC):
                        fillq.extend(make_outproj(qb - 1, ec))
                hA, hB = 2 * p, 2 * p + 1
                # C tiles rotate through 3 banks so the next pair's first
                # context matmul never waits on the previous evacuation copy
                CA = ps.tile([128, QB], f32, tag="C", bufs=3,
                             name=f"CA{qb}_{p}")
                CB = ps.tile([128, QB], f32, tag="C", bufs=3,
                             name=f"CB{qb}_{p}")

                def sc2(kc, S2t):
                    nc.tensor.matmul(
                        S2t[:, 0:QB], kts[p][0:64, 128 * kc:128 * (kc + 1)],
                        qts[p][0:64, q0:q0 + QB],
                        start=True, stop=True, tile_position=(0, 0))
                    nc.tensor.matmul(
                        S2t[:, QB:2 * QB],
                        kts[p][64:128, 128 * kc:128 * (kc + 1)],
                        qts[p][64:128, q0:q0 + QB],
                        start=True, stop=True, tile_position=(64, 0))

                S2 = ps.tile([128, 2 * QB], f32, tag="S2", bufs=2,
                             name=f"S2_{qb}_{p}_0")
                sc2(0, S2)
                for kc in range(NKC):
                    S2n = None
                    if kc + 1 < NKC:
                        S2n = ps.tile([128, 2 * QB], f32, tag="S2", bufs=2,
                                      name=f"S2_{qb}_{p}_{kc + 1}")
                        sc2(kc + 1, S2n)
                    if fillq:
                        fillq.popleft()()
                    if len(fillq) > 8:
                        fillq.popleft()()
                    P2 = pp.tile([128, 2 * QB], bf16, tag="P2",
                                 name=f"P2_{qb}_{p}_{kc}")
                    nc.scalar.activation(P2[:], S2[:], FT.Exp,
                                         bias=kb_t[:, kc:kc + 1], scale=1.0)
                    nc.tensor.matmul(CA[:], vhx[kc][:, hA, :], P2[:, 0:QB],
                                     start=(kc == 0), stop=(kc == NKC - 1))
                    nc.tensor.matmul(CB[:], vhx[kc][:, hB, :],
                                     P2[:, QB:2 * QB],
                                     start=(kc == 0), stop=(kc == NKC - 1))
                    S2 = S2n
                evac(qb * 6 + hA, CA)
                evac(qb * 6 + hB, CB)

        # flush remaining output-projection fillers
        while fillq:
            fillq.popleft()()
        psa.__exit__(None, None, None)
        qkv.__exit__(None, None, None)

        # ---- output projection tail: last q-block. mm-major across 6 PSUM
        # banks so the czT[2]-dependent matmuls start 12 matmuls in, hiding
        # the last normalization chain's latency.
        pso = tc.tile_pool(name="pso", bufs=1, space="PSUM")
        ps = pso.__enter__()
        t0 = (NQB - 1) * QB
        pos = [ps.tile([128, QB], f32, tag=f"po{ec}", name=f"pot{ec}")
               for ec in range(NEC)]
        for mm in range(NMC):
            for ec in range(NEC):
                nc.tensor.matmul(pos[ec][:],
                                 wo_t[mm][:, 128 * ec:128 * (ec + 1)],
                                 czT[mm][:, t0:t0 + QB],
                                 start=(mm == 0), stop=(mm == NMC - 1))
        for ec in range(NEC):
            ot = ost.tile([128, QB], f32, tag="ot", name=f"ott{ec}")
            # split the tail evacuations across DVE and Act (both idle now),
            # and the final DMAs across both hwdge engines (SP + Act) so the
            # output-queue drain overlaps instead of serializing
            if ec % 2 == 0:
                nc.vector.tensor_copy(ot[:], pos[ec][:])
                nc.scalar.dma_start(oT[128 * ec:128 * (ec + 1), t0:t0 + QB],
                                    ot[:])
            else:
                nc.scalar.copy(ot[:], pos[ec][:])
                nc.sync.dma_start(oT[128 * ec:128 * (ec + 1), t0:t0 + QB],
                                  ot[:])
        pso.__exit__(None, None, None)

    nc.compile()
    return nc


def _numpy_fallback(q, k, v, mask, Wq, bq, Wk, bk, Wv, bv, Wo, bo):
    B, Sq, _ = q.shape
    qh = (q @ Wq + bq).reshape(B, Sq, H, D).transpose(0, 2, 1, 3)
    kh = (k @ Wk + bk).reshape(B, -1, H, D).transpose(0, 2, 1, 3)
    vh = (v @ Wv + bv).reshape(B, -1, H, D).transpose(0, 2, 1, 3)
    s = np.einsum("bhqd,bhkd->bhqk", qh, kh) / np.sqrt(np.float32(D))
    s = s + np.where(mask == 0, np.float32(-1e9), np.float32(0))[:, None, None, :]
    s = s - s.max(-1, keepdims=True)
    w = np.exp(s)
    w = w / w.sum(-1, keepdims=True)
    ctx = np.einsum("bhqk,bhkd->bqhd", w, vh).reshape(B, Sq, E)
    return (ctx @ Wo + bo).astype(np.float32)


def kernel(q, k, v, mask, Wq, bq, Wk, bk, Wv, bv, Wo, bo):
    global _LAST
    q = np.asarray(q, np.float32)
    k = np.asarray(k, np.float32)
    v = np.asarray(v, np.float32)
    mask = np.asarray(mask)
    Wq = np.asarray(Wq, np.float32)
    bq = np.asarray(bq, np.float32)
    Wk = np.asarray(Wk, np.float32)
    bk = np.asarray(bk, np.float32)
    Wv = np.asarray(Wv, np.float32)
    bv = np.asarray(bv, np.float32)
    Wo = np.asarray(Wo, np.float32)
    bo = np.asarray(bo, np.float32)

    B, S_q, _ = q.shape
    idxs = [np.flatnonzero(mask[b]) for b in range(B)]
    ns = [len(ix) for ix in idxs]
    if min(ns) == 0 or B * 2 != N_CORES or S_q % 512 != 0:
        return _numpy_fallback(q, k, v, mask, Wq, bq, Wk, bk, Wv, bv, Wo, bo)

    S_pad = max(128, ((max(ns) + 127) // 128) * 128)
    NKC = S_pad // 128
    NMC = HALF // 128

    key = (S_q, S_pad)
    if key not in _CACHE:
        _CACHE[key] = _build(S_q, S_pad)
    nc = _CACHE[key]

    scale = np.float32(1.0 / np.sqrt(D))
    in_maps = []
    for c in range(N_CORES):
        b, j = divmod(c, 2)
        cols = slice(j * HALF, (j + 1) * HALF)
        kc_ = np.zeros((S_pad, E), np.float32)
        kc_[:ns[b]] = k[b][idxs[b]]
        vc_ = np.zeros((S_pad, E), np.float32)
        vc_[:ns[b]] = v[b][idxs[b]]
        kb_vec = np.zeros(S_pad, np.float32)
        kb_vec[ns[b]:] = -30000.0
        in_maps.append({
            "qT": np.ascontiguousarray(q[b].T).astype(bf16_np),
            "kT": np.ascontiguousarray(kc_.T).astype(bf16_np),
            "vT": np.ascontiguousarray(vc_.T).astype(bf16_np),
            "wq": (Wq[:, cols] * scale).astype(bf16_np),
            "wk": np.ascontiguousarray(Wk[:, cols]).astype(bf16_np),
            "wv": np.ascontiguousarray(Wv[:, cols]).astype(bf16_np),
            "wo": np.ascontiguousarray(Wo[cols, :]).astype(bf16_np),
            "bq2": np.ascontiguousarray((bq[cols] * scale).reshape(NMC, 128).T),
            "bk2": np.ascontiguousarray(bk[cols].reshape(NMC, 128).T),
            "kbias": np.ascontiguousarray(kb_vec.reshape(NKC, 128).T),
        })

    from concourse.bass_utils import run_bass_kernel_spmd
    res = run_bass_kernel_spmd(nc, in_maps, list(range(N_CORES)))
    _LAST = res

    bo_eff = bo + bv @ Wo
    out = np.empty((B, S_q, E), np.float32)
    for b in range(B):
        out[b] = (res.results[2 * b]["oT"] + res.results[2 * b + 1]["oT"]).T
        out[b] += bo_eff
    return out
